# revision 23
# baseline (speedup 1.0000x reference)
"""Multi-level block-diagonal sparse attention (AttMLR) on 8 TRN2 NeuronCores.

Sharding: head-parallel — core c owns heads (2c, 2c+1). Each core:
  1. computes qT/kT (scaled, [d, t] layout) and v ([t, d] layout) for its heads
     from a replicated x^T and its slice of Wqkv,
  2. computes causal multi-level scores, exp (fused PSUM->SBUF), masks the
     diagonal tiles, and accumulates y^T = v.T @ p^T with a fused ones-column
     that yields the softmax denominator,
  3. AllToAll redistributes y^T pieces so core c holds all heads' dims for
     t-slice c, then computes out_slice = y_slice @ Wproj.
Host assembles the 8 [256, 1024] slices.

Matmul operands are bf16; accumulation, scores and normalization stay fp32.
SBUF tensors are split per DMA-chunk / per block so Tile's dependency tracking
stays fine-grained (a consumer only waits for the chunk it reads).

Level structure: RANKS [32, 16, 16] over head-dim prefixes [0:32), [32:48),
[48:64) with block sizes [2048, 1024, 512]. Blocks nest, so a (k_tile, q_block)
pair contracts over a prefix of the 64 dims: 64 if same 512-block, 48 if same
1024-block, else 32 (level-0 spans all of T). Per-level 1/(rank*3) scaling is
folded into Wq columns on the host (before bf16 quantization).
"""

import ml_dtypes
import numpy as np

import concourse.bass as bass
import concourse.mybir as mybir
from concourse import bacc
from concourse.bass_utils import run_bass_kernel_spmd
from concourse.tile import TileContext
from concourse.masks import make_identity

T = 2048
C = 1024
H = 16
D = 64
NCORES = 8
P = 128
NO = C // P          # 8 contraction chunks of 128
QB = 512             # q-block size (score-tile free dim)
NQB = T // QB        # 4 q-blocks
NKT = T // P         # 16 k-tiles
TS = T // NCORES     # 256, per-core output t-slice
F32 = mybir.dt.float32
BF16 = mybir.dt.bfloat16
NPBF16 = ml_dtypes.bfloat16
EXP = mybir.ActivationFunctionType.Exp

_CACHE = {}


def _ki(i, j):
    """Contraction depth for score tile (k_tile i, q_block j)."""
    if i // 4 == j:
        return 64
    if i // 8 == j // 2:
        return 48
    return 32


def _build():
    nc = bacc.Bacc(None, target_bir_lowering=False, num_devices=NCORES)

    xT = nc.declare_dram_parameter("xT", [P, NO, T], BF16, isOutput=False)
    wq = nc.declare_dram_parameter("wq", [P, NO, P], BF16, isOutput=False)
    wk = nc.declare_dram_parameter("wk", [P, NO, P], BF16, isOutput=False)
    wv = nc.declare_dram_parameter("wv", [P, NO, P], BF16, isOutput=False)
    wproj = nc.declare_dram_parameter("wproj", [P, NO, C], BF16, isOutput=False)
    masks = nc.declare_dram_parameter("masks", [P, 4, QB], BF16, isOutput=False)
    out = nc.declare_dram_parameter("out", [P, 2, C], F32, isOutput=True)

    with TileContext(nc) as tc:
        with (
            tc.tile_pool(name="persist", bufs=1) as persist,
            tc.tile_pool(name="pt", bufs=8) as ptp,
            tc.tile_pool(name="nrm", bufs=2) as nrm,
            tc.tile_pool(name="st4", bufs=2) as st4,
            tc.tile_pool(name="dram", bufs=1, space="DRAM") as dram,
        ):
            wq_sb = persist.tile([P, NO, P], BF16)
            wk_sb = persist.tile([P, NO, P], BF16)
            wv_sb = persist.tile([P, NO, P], BF16)
            wproj_sb = persist.tile([P, NO, C], BF16)
            masks_sb = persist.tile([P, 4, QB], BF16)
            ident = persist.tile([P, P], BF16)
            # chunked tensors -> fine-grained RAW deps
            xT_sb = [persist.tile([P, T], BF16, name=f"xT{o}") for o in range(NO)]
            qT_sb = [persist.tile([P, QB], BF16, name=f"qT{b}") for b in range(NQB)]
            kT_sb = [persist.tile([P, QB], BF16, name=f"kT{b}") for b in range(NQB)]
            vT_sb = [persist.tile([P, QB], BF16, name=f"vT{b}") for b in range(NQB)]
            # v in natural [t, d] layout; per t_tile a [128, 2, 65] whose last
            # column per head is 1.0 (softmax denominator row).
            v_sb = [persist.tile([P, 2, 65], BF16, name=f"v{i}") for i in range(NKT)]
            yT_sb = [persist.tile([P, QB], BF16, name=f"yT{b}") for b in range(NQB)]
            yTall = [persist.tile([P, TS], BF16, name=f"yA{s}") for s in range(NCORES)]

            # spread DMA issue across sequencers (~620ns per dma_start issue)
            nc.scalar.dma_start(wq_sb[:], wq[:])
            nc.sync.dma_start(wk_sb[:], wk[:])
            nc.gpsimd.dma_start(wv_sb[:], wv[:])
            issuers = (nc.sync, nc.scalar, nc.gpsimd)
            for o in range(NO):
                issuers[o % 3].dma_start(xT_sb[o][:], xT[:, o, :])
            for i in range(NKT):
                nc.gpsimd.memset(v_sb[i][:, :, 64], 1.0)
            make_identity(nc, ident[:])
            a2a_in = dram.tile([NCORES, P, TS], BF16)
            a2a_out = dram.tile([NCORES, P, TS], BF16)
            # phase-2/4-only loads: issue after the x chunks
            nc.sync.dma_start(masks_sb[:], masks[:])
            nc.sync.dma_start(wproj_sb[:], wproj[:])

            # PE warmup (HAM un-throttle) + ACT exp-table preload while the
            # input DMAs stream in; identity tile is produced on gpsimd early.
            with tc.tile_pool(name="warm", bufs=1, space="PSUM") as wps:
                wp = wps.tile([P, P], F32, tag="warm")
                for _ in range(36):
                    nc.tensor.matmul(wp[:], ident[:], ident[:], start=True, stop=True)
                wact = nrm.tile([1, 1], F32, tag="wact")
                nc.scalar.activation(wact[:], ident[0:1, 0:1], EXP)

            # ---- Phase 1: qT/kT/vT projections + v transpose ----
            with (
                tc.tile_pool(name="ps1", bufs=4, space="PSUM") as ps1,
                tc.tile_pool(name="ps1t", bufs=2, space="PSUM") as ps1t,
            ):
                def _vtrans(tb):
                    for tt in range(4 * tb, 4 * tb + 4):
                        pst = ps1t.tile([P, P], BF16, tag="vtr",
                                        name=f"pst{tt}")
                        nc.tensor.transpose(
                            pst[:], vT_sb[tb][:, bass.ts(tt - 4 * tb, P)], ident[:]
                        )
                        nc.vector.tensor_copy(
                            v_sb[tt][:, :, 0:64],
                            pst[:].rearrange("p (h d) -> p h d", h=2),
                        )

                for tb in range(NQB):
                    for w_sb, dst in (
                        (wk_sb, kT_sb[tb]),
                        (wq_sb, qT_sb[tb]),
                        (wv_sb, vT_sb[tb]),
                    ):
                        ps = ps1.tile([P, QB], F32, tag="proj")
                        for o in range(NO):
                            nc.tensor.matmul(
                                ps[:],
                                w_sb[:, o, :],
                                xT_sb[o][:, bass.ts(tb, QB)],
                                start=(o == 0),
                                stop=(o == NO - 1),
                            )
                        nc.vector.tensor_copy(dst[:], ps[:])
                    if tb > 0:
                        _vtrans(tb - 1)
                _vtrans(NQB - 1)

            # ---- Phase 2: scores -> exp -> mask -> y^T accumulation ----
            # j descending: the final pieces (j=0) have the shortest drain, so
            # the AllToAll starts sooner after the last matmul.
            with (
                tc.tile_pool(name="ps2s", bufs=2, space="PSUM") as ps2s,
                tc.tile_pool(name="ps2y", bufs=2, space="PSUM") as ps2y,
            ):
                for j in range(NQB):
                    nkt = 4 * j + 4  # causal k-tiles for this q-block (even)
                    yps = [
                        ps2y.tile([65, QB], F32, tag=f"yps{h}", name=f"yps{h}_{j}")
                        for h in range(2)
                    ]
                    prev = None  # deferred av matmuls: (ptt, pair)
                    for pair in range(nkt // 2):
                        sps = [
                            ps2s.tile([P, 2 * QB], F32, tag="sps",
                                      name=f"sps{hh}_{j}_{pair}")
                            for hh in range(2)
                        ]
                        ptt = [
                            ptp.tile([P, 2 * QB], BF16, tag="pt",
                                     name=f"pt{hh}_{j}_{pair}")
                            for hh in range(2)
                        ]
                        for half in range(2):
                            i = 2 * pair + half
                            ki = _ki(i, j)
                            for h in range(2):
                                nc.tensor.matmul(
                                    sps[h][:, bass.ts(half, QB)],
                                    kT_sb[i // 4][h * D : h * D + ki,
                                                  bass.ts(i % 4, P)],
                                    qT_sb[j][h * D : h * D + ki, :],
                                    start=True,
                                    stop=True,
                                    tile_position=(h * D, 0),
                                )
                        for h in range(2):
                            nc.scalar.activation(ptt[h][:], sps[h][:], EXP)
                        for h in range(2):
                            for half in range(2):
                                i = 2 * pair + half
                                if i >= 4 * j:
                                    nc.vector.tensor_mul(
                                        ptt[h][:, bass.ts(half, QB)],
                                        ptt[h][:, bass.ts(half, QB)],
                                        masks_sb[:, i - 4 * j, :],
                                    )
                        # emit the PREVIOUS pair's av matmuls now: the PE
                        # stream interleaves next-pair scores with these, so
                        # the PE isn't head-of-line blocked on this pair's exp.
                        if prev is not None:
                            pptt, ppair = prev
                            for h in range(2):
                                for half in range(2):
                                    i = 2 * ppair + half
                                    nc.tensor.matmul(
                                        yps[h][:],
                                        v_sb[i][:, h, :],
                                        pptt[h][:, bass.ts(half, QB)],
                                        start=(i == 0),
                                        stop=False,
                                    )
                        prev = (ptt, pair)
                    pptt, ppair = prev
                    for h in range(2):
                        for half in range(2):
                            i = 2 * ppair + half
                            nc.tensor.matmul(
                                yps[h][:],
                                v_sb[i][:, h, :],
                                pptt[h][:, bass.ts(half, QB)],
                                start=(i == 0),
                                stop=(i == nkt - 1),
                            )
                    for h in range(2):
                        # one copy releases the PSUM bank; normalize from SBUF
                        yn = nrm.tile([65, QB], F32, tag="yn", name=f"yn{h}_{j}")
                        nc.vector.tensor_copy(yn[:], yps[h][:])
                        den = nrm.tile([1, QB], F32, tag="den", name=f"den{h}_{j}")
                        nc.vector.tensor_copy(den[:], yn[64:65, :])
                        rec = nrm.tile([1, QB], F32, tag="rec", name=f"rec{h}_{j}")
                        nc.vector.reciprocal_approx_fast(rec[:], den[:])
                        bc = nrm.tile([64, QB], F32, tag="bc", name=f"bc{h}_{j}")
                        nc.gpsimd.partition_broadcast(bc[:], rec[:])
                        with nc.allow_low_precision(reason="bf16 y for comms"):
                            nc.vector.tensor_mul(
                                yT_sb[j][h * D : (h + 1) * D, :],
                                yn[0:64, :],
                                bc[:],
                            )
                    for half in range(2):
                        nc.sync.dma_start(
                            a2a_in[2 * j + half],
                            yT_sb[j][:, bass.ts(half, TS)],
                        )

            # ---- Phase 3: AllToAll of y^T pieces (inputs DMA'd per-j above) ----
            nc.gpsimd.collective_compute(
                "AllToAll",
                mybir.AluOpType.bypass,
                replica_groups=[list(range(NCORES))],
                ins=[a2a_in.opt()],
                outs=[a2a_out.opt()],
            )
            for s in range(NCORES):
                nc.sync.dma_start(yTall[s][:], a2a_out[s])

            # ---- Phase 4: out_slice = y_slice @ Wproj ----
            with tc.tile_pool(name="ps4", bufs=2, space="PSUM") as ps4:
                for tt in range(2):
                    for nb in range(2):
                        pso = ps4.tile([P, QB], F32, tag="pso")
                        for o in range(NO):
                            nc.tensor.matmul(
                                pso[:],
                                yTall[o][:, bass.ts(tt, P)],
                                wproj_sb[:, o, bass.ts(nb, QB)],
                                start=(o == 0),
                                stop=(o == NO - 1),
                            )
                        stage = st4.tile([P, QB], F32, tag="stage",
                                         name=f"stage{tt}_{nb}")
                        nc.scalar.copy(stage[:], pso[:])
                        nc.sync.dma_start(out[:, tt, bass.ts(nb, QB)], stage[:])

    nc.compile()
    return nc


def _prep_inputs(x, Wqkv, Wproj):
    x2 = np.ascontiguousarray(x.reshape(T, C))
    xT = np.ascontiguousarray(x2.T)                       # [C, T]
    xT_a = np.ascontiguousarray(
        xT.reshape(NO, P, T).transpose(1, 0, 2)
    ).astype(NPBF16)

    # per-dim scale folded into Wq: 1/(rank*3) by level of (d % 64)
    colscale = np.where(np.arange(P) % D < 32, 1.0 / 96, 1.0 / 48).astype(
        np.float32
    )

    wproj_a = np.ascontiguousarray(
        Wproj.reshape(NO, P, C).transpose(1, 0, 2)
    ).astype(NPBF16)

    kp = np.arange(P)[:, None]
    qf = np.arange(QB)[None, :]
    masks = np.stack(
        [(qf >= kp + P * d).astype(np.float32) for d in range(4)], axis=0
    )
    masks_a = np.ascontiguousarray(masks.transpose(1, 0, 2)).astype(NPBF16)

    in_maps = []
    for c in range(NCORES):
        cs = slice(P * c, P * (c + 1))
        wq_c = Wqkv[:, cs] * colscale[None, :]
        wk_c = Wqkv[:, C : 2 * C][:, cs]
        wv_c = Wqkv[:, 2 * C :][:, cs]
        in_maps.append(
            {
                "xT": xT_a,
                "wq": np.ascontiguousarray(
                    wq_c.reshape(NO, P, P).transpose(1, 0, 2)
                ).astype(NPBF16),
                "wk": np.ascontiguousarray(
                    wk_c.reshape(NO, P, P).transpose(1, 0, 2)
                ).astype(NPBF16),
                "wv": np.ascontiguousarray(
                    wv_c.reshape(NO, P, P).transpose(1, 0, 2)
                ).astype(NPBF16),
                "wproj": wproj_a,
                "masks": masks_a,
            }
        )
    return in_maps


def kernel(x, Wqkv, Wproj, _trace=False):
    x = np.asarray(x, np.float32)
    Wqkv = np.asarray(Wqkv, np.float32)
    Wproj = np.asarray(Wproj, np.float32)

    if "nc" not in _CACHE:
        _CACHE["nc"] = _build()
    nc = _CACHE["nc"]

    in_maps = _prep_inputs(x, Wqkv, Wproj)
    res = run_bass_kernel_spmd(nc, in_maps, list(range(NCORES)), trace=_trace)
    _CACHE["last_result"] = res

    full = np.empty((T, C), np.float32)
    for c in range(NCORES):
        oc = res.results[c]["out"]  # [128, 2, 1024]
        full[2 * P * c : 2 * P * (c + 1)] = oc.transpose(1, 0, 2).reshape(
            2 * P, C
        )
    return full.reshape(1, T, C)


# revision 24
# speedup vs baseline: 1.1757x; 1.1757x over previous
"""Multi-level block-diagonal sparse attention (AttMLR) on 8 TRN2 NeuronCores.

Sharding: head-parallel — core c owns heads (2c, 2c+1). Each core:
  1. computes qT/kT (scaled, [d, t] layout) and v ([t, d] layout) for its heads
     from a replicated x^T and its slice of Wqkv,
  2. computes causal multi-level scores, exp (fused PSUM->SBUF), masks the
     diagonal tiles, and accumulates y^T = v.T @ p^T with a fused ones-column
     that yields the softmax denominator,
  3. AllToAll redistributes y^T pieces so core c holds all heads' dims for
     t-slice c, then computes out_slice = y_slice @ Wproj.
Host assembles the 8 [256, 1024] slices.

Matmul operands are bf16; accumulation, scores and normalization stay fp32.
SBUF tensors are split per DMA-chunk / per block so Tile's dependency tracking
stays fine-grained (a consumer only waits for the chunk it reads).

Level structure: RANKS [32, 16, 16] over head-dim prefixes [0:32), [32:48),
[48:64) with block sizes [2048, 1024, 512]. Blocks nest, so a (k_tile, q_block)
pair contracts over a prefix of the 64 dims: 64 if same 512-block, 48 if same
1024-block, else 32 (level-0 spans all of T). Per-level 1/(rank*3) scaling is
folded into Wq columns on the host (before bf16 quantization).
"""

import ml_dtypes
import numpy as np

import concourse.bass as bass
import concourse.mybir as mybir
from concourse import bacc
from concourse.bass_utils import run_bass_kernel_spmd
from concourse.tile import TileContext
from concourse.masks import make_identity

T = 2048
C = 1024
H = 16
D = 64
NCORES = 8
P = 128
NO = C // P          # 8 contraction chunks of 128
QB = 512             # q-block size (score-tile free dim)
NQB = T // QB        # 4 q-blocks
NKT = T // P         # 16 k-tiles
TS = T // NCORES     # 256, per-core output t-slice
F32 = mybir.dt.float32
BF16 = mybir.dt.bfloat16
NPBF16 = ml_dtypes.bfloat16
EXP = mybir.ActivationFunctionType.Exp

_CACHE = {}


def _ki(i, j):
    """Contraction depth for score tile (k_tile i, q_block j)."""
    if i // 4 == j:
        return 64
    if i // 8 == j // 2:
        return 48
    return 32


def _build():
    nc = bacc.Bacc(None, target_bir_lowering=False, num_devices=NCORES)

    xT = nc.declare_dram_parameter("xT", [P, NO, T], BF16, isOutput=False)
    wq = nc.declare_dram_parameter("wq", [P, NO, P], BF16, isOutput=False)
    wk = nc.declare_dram_parameter("wk", [P, NO, P], BF16, isOutput=False)
    wv = nc.declare_dram_parameter("wv", [P, NO, P], BF16, isOutput=False)
    wproj = nc.declare_dram_parameter("wproj", [P, NO, C], BF16, isOutput=False)
    masks = nc.declare_dram_parameter("masks", [P, 4, QB], BF16, isOutput=False)
    out = nc.declare_dram_parameter("out", [P, 2, C], F32, isOutput=True)

    with TileContext(nc) as tc:
        with (
            tc.tile_pool(name="persist", bufs=1) as persist,
            tc.tile_pool(name="pt", bufs=8) as ptp,
            tc.tile_pool(name="nrm", bufs=2) as nrm,
            tc.tile_pool(name="st4", bufs=2) as st4,
            tc.tile_pool(name="dram", bufs=1, space="DRAM") as dram,
        ):
            wq_sb = persist.tile([P, NO, P], BF16)
            wk_sb = persist.tile([P, NO, P], BF16)
            wv_sb = persist.tile([P, NO, P], BF16)
            wproj_sb = persist.tile([P, NO, C], BF16)
            masks_sb = persist.tile([P, 4, QB], BF16)
            ident = persist.tile([P, P], BF16)
            # chunked tensors -> fine-grained RAW deps
            xT_sb = [persist.tile([P, T], BF16, name=f"xT{o}") for o in range(NO)]
            qT_sb = [persist.tile([P, QB], BF16, name=f"qT{b}") for b in range(NQB)]
            kT_sb = [persist.tile([P, QB], BF16, name=f"kT{b}") for b in range(NQB)]
            vT_sb = [persist.tile([P, QB], BF16, name=f"vT{b}") for b in range(NQB)]
            # v in natural [t, d] layout; per t_tile a [128, 2, 65] whose last
            # column per head is 1.0 (softmax denominator row).
            v_sb = [persist.tile([P, 2, 65], BF16, name=f"v{i}") for i in range(NKT)]
            yT_sb = [persist.tile([P, QB], BF16, name=f"yT{b}") for b in range(NQB)]
            yTall = [persist.tile([P, TS], BF16, name=f"yA{s}") for s in range(NCORES)]

            # spread DMA issue across sequencers (~620ns per dma_start issue)
            nc.scalar.dma_start(wq_sb[:], wq[:])
            nc.sync.dma_start(wk_sb[:], wk[:])
            nc.gpsimd.dma_start(wv_sb[:], wv[:])
            issuers = (nc.sync, nc.scalar, nc.gpsimd)
            for o in range(NO):
                issuers[o % 3].dma_start(xT_sb[o][:], xT[:, o, :])
            for i in range(NKT):
                nc.gpsimd.memset(v_sb[i][:, :, 64], 1.0)
            make_identity(nc, ident[:])
            a2a_in = dram.tile([NCORES, P, TS], BF16)
            a2a_out = dram.tile([NCORES, P, TS], BF16)
            # phase-2/4-only loads: issue after the x chunks
            nc.sync.dma_start(masks_sb[:], masks[:])
            nc.sync.dma_start(wproj_sb[:], wproj[:])

            # PE warmup (HAM un-throttle) + ACT exp-table preload while the
            # input DMAs stream in; identity tile is produced on gpsimd early.
            with tc.tile_pool(name="warm", bufs=1, space="PSUM") as wps:
                wp = wps.tile([P, P], F32, tag="warm")
                for _ in range(36):
                    nc.tensor.matmul(wp[:], ident[:], ident[:], start=True, stop=True)
                wact = nrm.tile([1, 1], F32, tag="wact")
                nc.scalar.activation(wact[:], ident[0:1, 0:1], EXP)

            # ---- Phase 1: qT/kT/vT projections + v transpose ----
            with (
                tc.tile_pool(name="ps1", bufs=4, space="PSUM") as ps1,
                tc.tile_pool(name="ps1t", bufs=2, space="PSUM") as ps1t,
            ):
                def _vtrans(tb):
                    for tt in range(4 * tb, 4 * tb + 4):
                        pst = ps1t.tile([P, P], BF16, tag="vtr",
                                        name=f"pst{tt}")
                        nc.tensor.transpose(
                            pst[:], vT_sb[tb][:, bass.ts(tt - 4 * tb, P)], ident[:]
                        )
                        nc.vector.tensor_copy(
                            v_sb[tt][:, :, 0:64],
                            pst[:].rearrange("p (h d) -> p h d", h=2),
                        )

                for tb in range(NQB):
                    for w_sb, dst in (
                        (wk_sb, kT_sb[tb]),
                        (wq_sb, qT_sb[tb]),
                        (wv_sb, vT_sb[tb]),
                    ):
                        ps = ps1.tile([P, QB], F32, tag="proj")
                        for o in range(NO):
                            nc.tensor.matmul(
                                ps[:],
                                w_sb[:, o, :],
                                xT_sb[o][:, bass.ts(tb, QB)],
                                start=(o == 0),
                                stop=(o == NO - 1),
                            )
                        nc.vector.tensor_copy(dst[:], ps[:])
                    if tb > 0:
                        _vtrans(tb - 1)
                _vtrans(NQB - 1)

            # ---- Phase 2: scores -> exp -> mask -> y^T accumulation ----
            # j descending: the final pieces (j=0) have the shortest drain, so
            # the AllToAll starts sooner after the last matmul.
            with (
                tc.tile_pool(name="ps2s", bufs=3, space="PSUM") as ps2s,
                tc.tile_pool(name="ps2y", bufs=1, space="PSUM") as ps2y,
            ):
                for j in range(NQB):
                    nkt = 4 * j + 4  # causal k-tiles for this q-block (even)
                    yps = [
                        ps2y.tile([65, QB], F32, tag=f"yps{h}", name=f"yps{h}_{j}")
                        for h in range(2)
                    ]
                    prev = None  # deferred av matmuls: (ptt, pair)
                    for pair in range(nkt // 2):
                        sps = [
                            ps2s.tile([P, 2 * QB], F32, tag="sps",
                                      name=f"sps{hh}_{j}_{pair}")
                            for hh in range(2)
                        ]
                        ptt = [
                            ptp.tile([P, 2 * QB], BF16, tag="pt",
                                     name=f"pt{hh}_{j}_{pair}")
                            for hh in range(2)
                        ]
                        for half in range(2):
                            i = 2 * pair + half
                            ki = _ki(i, j)
                            for h in range(2):
                                nc.tensor.matmul(
                                    sps[h][:, bass.ts(half, QB)],
                                    kT_sb[i // 4][h * D : h * D + ki,
                                                  bass.ts(i % 4, P)],
                                    qT_sb[j][h * D : h * D + ki, :],
                                    start=True,
                                    stop=True,
                                    tile_position=(h * D, 0),
                                )
                        for h in range(2):
                            nc.scalar.activation(ptt[h][:], sps[h][:], EXP)
                        for h in range(2):
                            for half in range(2):
                                i = 2 * pair + half
                                if i >= 4 * j:
                                    nc.vector.tensor_mul(
                                        ptt[h][:, bass.ts(half, QB)],
                                        ptt[h][:, bass.ts(half, QB)],
                                        masks_sb[:, i - 4 * j, :],
                                    )
                        # emit the PREVIOUS pair's av matmuls now: the PE
                        # stream interleaves next-pair scores with these, so
                        # the PE isn't head-of-line blocked on this pair's exp.
                        if prev is not None:
                            pptt, ppair = prev
                            for h in range(2):
                                for half in range(2):
                                    i = 2 * ppair + half
                                    nc.tensor.matmul(
                                        yps[h][:],
                                        v_sb[i][:, h, :],
                                        pptt[h][:, bass.ts(half, QB)],
                                        start=(i == 0),
                                        stop=False,
                                    )
                        prev = (ptt, pair)
                    pptt, ppair = prev
                    for h in range(2):
                        for half in range(2):
                            i = 2 * ppair + half
                            nc.tensor.matmul(
                                yps[h][:],
                                v_sb[i][:, h, :],
                                pptt[h][:, bass.ts(half, QB)],
                                start=(i == 0),
                                stop=(i == nkt - 1),
                            )
                    for h in range(2):
                        # one copy releases the PSUM bank; normalize from SBUF
                        yn = nrm.tile([65, QB], F32, tag="yn", name=f"yn{h}_{j}")
                        nc.vector.tensor_copy(yn[:], yps[h][:])
                        den = nrm.tile([1, QB], F32, tag="den", name=f"den{h}_{j}")
                        nc.vector.tensor_copy(den[:], yn[64:65, :])
                        rec = nrm.tile([1, QB], F32, tag="rec", name=f"rec{h}_{j}")
                        nc.vector.reciprocal_approx_fast(rec[:], den[:])
                        bc = nrm.tile([64, QB], F32, tag="bc", name=f"bc{h}_{j}")
                        nc.gpsimd.partition_broadcast(bc[:], rec[:])
                        with nc.allow_low_precision(reason="bf16 y for comms"):
                            nc.vector.tensor_mul(
                                yT_sb[j][h * D : (h + 1) * D, :],
                                yn[0:64, :],
                                bc[:],
                            )
                    for half in range(2):
                        nc.sync.dma_start(
                            a2a_in[2 * j + half],
                            yT_sb[j][:, bass.ts(half, TS)],
                        )

            # ---- Phase 3: AllToAll of y^T pieces (inputs DMA'd per-j above) ----
            nc.gpsimd.collective_compute(
                "AllToAll",
                mybir.AluOpType.bypass,
                replica_groups=[list(range(NCORES))],
                ins=[a2a_in.opt()],
                outs=[a2a_out.opt()],
            )
            for s in range(NCORES):
                nc.sync.dma_start(yTall[s][:], a2a_out[s])

            # ---- Phase 4: out_slice = y_slice @ Wproj ----
            with tc.tile_pool(name="ps4", bufs=2, space="PSUM") as ps4:
                for tt in range(2):
                    for nb in range(2):
                        pso = ps4.tile([P, QB], F32, tag="pso")
                        for o in range(NO):
                            nc.tensor.matmul(
                                pso[:],
                                yTall[o][:, bass.ts(tt, P)],
                                wproj_sb[:, o, bass.ts(nb, QB)],
                                start=(o == 0),
                                stop=(o == NO - 1),
                            )
                        stage = st4.tile([P, QB], F32, tag="stage",
                                         name=f"stage{tt}_{nb}")
                        nc.scalar.copy(stage[:], pso[:])
                        nc.sync.dma_start(out[:, tt, bass.ts(nb, QB)], stage[:])

    nc.compile()
    return nc


def _prep_inputs(x, Wqkv, Wproj):
    x2 = np.ascontiguousarray(x.reshape(T, C))
    xT = np.ascontiguousarray(x2.T)                       # [C, T]
    xT_a = np.ascontiguousarray(
        xT.reshape(NO, P, T).transpose(1, 0, 2)
    ).astype(NPBF16)

    # per-dim scale folded into Wq: 1/(rank*3) by level of (d % 64)
    colscale = np.where(np.arange(P) % D < 32, 1.0 / 96, 1.0 / 48).astype(
        np.float32
    )

    wproj_a = np.ascontiguousarray(
        Wproj.reshape(NO, P, C).transpose(1, 0, 2)
    ).astype(NPBF16)

    kp = np.arange(P)[:, None]
    qf = np.arange(QB)[None, :]
    masks = np.stack(
        [(qf >= kp + P * d).astype(np.float32) for d in range(4)], axis=0
    )
    masks_a = np.ascontiguousarray(masks.transpose(1, 0, 2)).astype(NPBF16)

    in_maps = []
    for c in range(NCORES):
        cs = slice(P * c, P * (c + 1))
        wq_c = Wqkv[:, cs] * colscale[None, :]
        wk_c = Wqkv[:, C : 2 * C][:, cs]
        wv_c = Wqkv[:, 2 * C :][:, cs]
        in_maps.append(
            {
                "xT": xT_a,
                "wq": np.ascontiguousarray(
                    wq_c.reshape(NO, P, P).transpose(1, 0, 2)
                ).astype(NPBF16),
                "wk": np.ascontiguousarray(
                    wk_c.reshape(NO, P, P).transpose(1, 0, 2)
                ).astype(NPBF16),
                "wv": np.ascontiguousarray(
                    wv_c.reshape(NO, P, P).transpose(1, 0, 2)
                ).astype(NPBF16),
                "wproj": wproj_a,
                "masks": masks_a,
            }
        )
    return in_maps


def kernel(x, Wqkv, Wproj, _trace=False):
    x = np.asarray(x, np.float32)
    Wqkv = np.asarray(Wqkv, np.float32)
    Wproj = np.asarray(Wproj, np.float32)

    if "nc" not in _CACHE:
        _CACHE["nc"] = _build()
    nc = _CACHE["nc"]

    in_maps = _prep_inputs(x, Wqkv, Wproj)
    res = run_bass_kernel_spmd(nc, in_maps, list(range(NCORES)), trace=_trace)
    _CACHE["last_result"] = res

    full = np.empty((T, C), np.float32)
    for c in range(NCORES):
        oc = res.results[c]["out"]  # [128, 2, 1024]
        full[2 * P * c : 2 * P * (c + 1)] = oc.transpose(1, 0, 2).reshape(
            2 * P, C
        )
    return full.reshape(1, T, C)


# revision 25
# speedup vs baseline: 1.1766x; 1.0007x over previous
"""Multi-level block-diagonal sparse attention (AttMLR) on 8 TRN2 NeuronCores.

Sharding: head-parallel — core c owns heads (2c, 2c+1). Each core:
  1. computes qT/kT (scaled, [d, t] layout) and v ([t, d] layout) for its heads
     from a replicated x^T and its slice of Wqkv,
  2. computes causal multi-level scores, exp (fused PSUM->SBUF), masks the
     diagonal tiles, and accumulates y^T = v.T @ p^T with a fused ones-column
     that yields the softmax denominator,
  3. AllToAll redistributes y^T pieces so core c holds all heads' dims for
     t-slice c, then computes out_slice = y_slice @ Wproj.
Host assembles the 8 [256, 1024] slices.

Matmul operands are bf16; accumulation, scores and normalization stay fp32.
SBUF tensors are split per DMA-chunk / per block so Tile's dependency tracking
stays fine-grained (a consumer only waits for the chunk it reads).

Level structure: RANKS [32, 16, 16] over head-dim prefixes [0:32), [32:48),
[48:64) with block sizes [2048, 1024, 512]. Blocks nest, so a (k_tile, q_block)
pair contracts over a prefix of the 64 dims: 64 if same 512-block, 48 if same
1024-block, else 32 (level-0 spans all of T). Per-level 1/(rank*3) scaling is
folded into Wq columns on the host (before bf16 quantization).
"""

import ml_dtypes
import numpy as np

import concourse.bass as bass
import concourse.mybir as mybir
from concourse import bacc
from concourse.bass_utils import run_bass_kernel_spmd
from concourse.tile import TileContext
from concourse.masks import make_identity

T = 2048
C = 1024
H = 16
D = 64
NCORES = 8
P = 128
NO = C // P          # 8 contraction chunks of 128
QB = 512             # q-block size (score-tile free dim)
NQB = T // QB        # 4 q-blocks
NKT = T // P         # 16 k-tiles
TS = T // NCORES     # 256, per-core output t-slice
F32 = mybir.dt.float32
BF16 = mybir.dt.bfloat16
NPBF16 = ml_dtypes.bfloat16
EXP = mybir.ActivationFunctionType.Exp

_CACHE = {}


def _ki(i, j):
    """Contraction depth for score tile (k_tile i, q_block j)."""
    if i // 4 == j:
        return 64
    if i // 8 == j // 2:
        return 48
    return 32


def _build():
    nc = bacc.Bacc(None, target_bir_lowering=False, num_devices=NCORES)

    xT = nc.declare_dram_parameter("xT", [P, NO, T], BF16, isOutput=False)
    wq = nc.declare_dram_parameter("wq", [P, NO, P], BF16, isOutput=False)
    wk = nc.declare_dram_parameter("wk", [P, NO, P], BF16, isOutput=False)
    wv = nc.declare_dram_parameter("wv", [P, NO, P], BF16, isOutput=False)
    wproj = nc.declare_dram_parameter("wproj", [P, NO, C], BF16, isOutput=False)
    masks = nc.declare_dram_parameter("masks", [P, 4, QB], BF16, isOutput=False)
    out = nc.declare_dram_parameter("out", [P, 2, C], F32, isOutput=True)

    with TileContext(nc) as tc:
        with (
            tc.tile_pool(name="persist", bufs=1) as persist,
            tc.tile_pool(name="pt", bufs=8) as ptp,
            tc.tile_pool(name="nrm", bufs=2) as nrm,
            tc.tile_pool(name="st4", bufs=2) as st4,
            tc.tile_pool(name="dram", bufs=1, space="DRAM") as dram,
        ):
            wq_sb = persist.tile([P, NO, P], BF16)
            wk_sb = persist.tile([P, NO, P], BF16)
            wv_sb = persist.tile([P, NO, P], BF16)
            wproj_sb = persist.tile([P, NO, C], BF16)
            masks_sb = persist.tile([P, 4, QB], BF16)
            ident = persist.tile([P, P], BF16)
            # chunked tensors -> fine-grained RAW deps
            xT_sb = [persist.tile([P, T], BF16, name=f"xT{o}") for o in range(NO)]
            qT_sb = [persist.tile([P, QB], BF16, name=f"qT{b}") for b in range(NQB)]
            kT_sb = [persist.tile([P, QB], BF16, name=f"kT{b}") for b in range(NQB)]
            vT_sb = [persist.tile([P, QB], BF16, name=f"vT{b}") for b in range(NQB)]
            # v in natural [t, d] layout; per t_tile a [128, 2, 65] whose last
            # column per head is 1.0 (softmax denominator row).
            v_sb = [persist.tile([P, 2, 65], BF16, name=f"v{i}") for i in range(NKT)]
            yT_sb = [persist.tile([P, QB], BF16, name=f"yT{b}") for b in range(NQB)]
            yTall = persist.tile([P, NCORES, TS], BF16)

            # spread DMA issue across sequencers (~620ns per dma_start issue)
            nc.scalar.dma_start(wq_sb[:], wq[:])
            nc.sync.dma_start(wk_sb[:], wk[:])
            nc.gpsimd.dma_start(wv_sb[:], wv[:])
            issuers = (nc.sync, nc.scalar, nc.gpsimd)
            for o in range(NO):
                issuers[o % 3].dma_start(xT_sb[o][:], xT[:, o, :])
            for i in range(NKT):
                nc.gpsimd.memset(v_sb[i][:, :, 64], 1.0)
            make_identity(nc, ident[:])
            a2a_in = dram.tile([NCORES, P, TS], BF16)
            a2a_out = dram.tile([NCORES, P, TS], BF16)
            # phase-2/4-only loads: issue after the x chunks
            nc.sync.dma_start(masks_sb[:], masks[:])
            nc.sync.dma_start(wproj_sb[:], wproj[:])

            # PE warmup (HAM un-throttle) + ACT exp-table preload while the
            # input DMAs stream in; identity tile is produced on gpsimd early.
            with tc.tile_pool(name="warm", bufs=1, space="PSUM") as wps:
                wp = wps.tile([P, P], F32, tag="warm")
                for _ in range(36):
                    nc.tensor.matmul(wp[:], ident[:], ident[:], start=True, stop=True)
                wact = nrm.tile([1, 1], F32, tag="wact")
                nc.scalar.activation(wact[:], ident[0:1, 0:1], EXP)

            # ---- Phase 1: qT/kT/vT projections + v transpose ----
            with (
                tc.tile_pool(name="ps1", bufs=4, space="PSUM") as ps1,
                tc.tile_pool(name="ps1t", bufs=2, space="PSUM") as ps1t,
            ):
                def _vtrans(tb):
                    for tt in range(4 * tb, 4 * tb + 4):
                        pst = ps1t.tile([P, P], BF16, tag="vtr",
                                        name=f"pst{tt}")
                        nc.tensor.transpose(
                            pst[:], vT_sb[tb][:, bass.ts(tt - 4 * tb, P)], ident[:]
                        )
                        nc.vector.tensor_copy(
                            v_sb[tt][:, :, 0:64],
                            pst[:].rearrange("p (h d) -> p h d", h=2),
                        )

                for tb in range(NQB):
                    for w_sb, dst in (
                        (wk_sb, kT_sb[tb]),
                        (wq_sb, qT_sb[tb]),
                        (wv_sb, vT_sb[tb]),
                    ):
                        ps = ps1.tile([P, QB], F32, tag="proj")
                        for o in range(NO):
                            nc.tensor.matmul(
                                ps[:],
                                w_sb[:, o, :],
                                xT_sb[o][:, bass.ts(tb, QB)],
                                start=(o == 0),
                                stop=(o == NO - 1),
                            )
                        nc.vector.tensor_copy(dst[:], ps[:])
                    if tb > 0:
                        _vtrans(tb - 1)
                _vtrans(NQB - 1)

            # ---- Phase 2: scores -> exp -> mask -> y^T accumulation ----
            # j descending: the final pieces (j=0) have the shortest drain, so
            # the AllToAll starts sooner after the last matmul.
            with (
                tc.tile_pool(name="ps2s", bufs=3, space="PSUM") as ps2s,
                tc.tile_pool(name="ps2y", bufs=1, space="PSUM") as ps2y,
            ):
                for j in range(NQB):
                    nkt = 4 * j + 4  # causal k-tiles for this q-block (even)
                    yps = [
                        ps2y.tile([65, QB], F32, tag=f"yps{h}", name=f"yps{h}_{j}")
                        for h in range(2)
                    ]
                    prev = None  # deferred av matmuls: (ptt, pair)
                    for pair in range(nkt // 2):
                        sps = [
                            ps2s.tile([P, 2 * QB], F32, tag="sps",
                                      name=f"sps{hh}_{j}_{pair}")
                            for hh in range(2)
                        ]
                        ptt = [
                            ptp.tile([P, 2 * QB], BF16, tag="pt",
                                     name=f"pt{hh}_{j}_{pair}")
                            for hh in range(2)
                        ]
                        for half in range(2):
                            i = 2 * pair + half
                            ki = _ki(i, j)
                            for h in range(2):
                                nc.tensor.matmul(
                                    sps[h][:, bass.ts(half, QB)],
                                    kT_sb[i // 4][h * D : h * D + ki,
                                                  bass.ts(i % 4, P)],
                                    qT_sb[j][h * D : h * D + ki, :],
                                    start=True,
                                    stop=True,
                                    tile_position=(h * D, 0),
                                )
                        for h in range(2):
                            nc.scalar.activation(ptt[h][:], sps[h][:], EXP)
                        for h in range(2):
                            for half in range(2):
                                i = 2 * pair + half
                                if i >= 4 * j:
                                    nc.vector.tensor_mul(
                                        ptt[h][:, bass.ts(half, QB)],
                                        ptt[h][:, bass.ts(half, QB)],
                                        masks_sb[:, i - 4 * j, :],
                                    )
                        # emit the PREVIOUS pair's av matmuls now: the PE
                        # stream interleaves next-pair scores with these, so
                        # the PE isn't head-of-line blocked on this pair's exp.
                        if prev is not None:
                            pptt, ppair = prev
                            for h in range(2):
                                for half in range(2):
                                    i = 2 * ppair + half
                                    nc.tensor.matmul(
                                        yps[h][:],
                                        v_sb[i][:, h, :],
                                        pptt[h][:, bass.ts(half, QB)],
                                        start=(i == 0),
                                        stop=False,
                                    )
                        prev = (ptt, pair)
                    pptt, ppair = prev
                    for h in range(2):
                        for half in range(2):
                            i = 2 * ppair + half
                            nc.tensor.matmul(
                                yps[h][:],
                                v_sb[i][:, h, :],
                                pptt[h][:, bass.ts(half, QB)],
                                start=(i == 0),
                                stop=(i == nkt - 1),
                            )
                    for h in range(2):
                        # one copy releases the PSUM bank; normalize from SBUF
                        yn = nrm.tile([65, QB], F32, tag="yn", name=f"yn{h}_{j}")
                        nc.vector.tensor_copy(yn[:], yps[h][:])
                        den = nrm.tile([1, QB], F32, tag="den", name=f"den{h}_{j}")
                        nc.vector.tensor_copy(den[:], yn[64:65, :])
                        rec = nrm.tile([1, QB], F32, tag="rec", name=f"rec{h}_{j}")
                        nc.vector.reciprocal_approx_fast(rec[:], den[:])
                        bc = nrm.tile([64, QB], F32, tag="bc", name=f"bc{h}_{j}")
                        nc.gpsimd.partition_broadcast(bc[:], rec[:])
                        with nc.allow_low_precision(reason="bf16 y for comms"):
                            nc.vector.tensor_mul(
                                yT_sb[j][h * D : (h + 1) * D, :],
                                yn[0:64, :],
                                bc[:],
                            )
                    for half in range(2):
                        nc.sync.dma_start(
                            a2a_in[2 * j + half],
                            yT_sb[j][:, bass.ts(half, TS)],
                        )

            # ---- Phase 3: AllToAll of y^T pieces (inputs DMA'd per-j above) ----
            nc.gpsimd.collective_compute(
                "AllToAll",
                mybir.AluOpType.bypass,
                replica_groups=[list(range(NCORES))],
                ins=[a2a_in.opt()],
                outs=[a2a_out.opt()],
            )
            nc.sync.dma_start(yTall[:], a2a_out[:].rearrange("s p t -> p s t"))

            # ---- Phase 4: out_slice = y_slice @ Wproj ----
            with tc.tile_pool(name="ps4", bufs=2, space="PSUM") as ps4:
                for tt in range(2):
                    for nb in range(2):
                        pso = ps4.tile([P, QB], F32, tag="pso")
                        for o in range(NO):
                            nc.tensor.matmul(
                                pso[:],
                                yTall[:, o, bass.ts(tt, P)],
                                wproj_sb[:, o, bass.ts(nb, QB)],
                                start=(o == 0),
                                stop=(o == NO - 1),
                            )
                        stage = st4.tile([P, QB], F32, tag="stage",
                                         name=f"stage{tt}_{nb}")
                        nc.scalar.copy(stage[:], pso[:])
                        nc.scalar.dma_start(out[:, tt, bass.ts(nb, QB)], stage[:])

    nc.compile()
    return nc


def _prep_inputs(x, Wqkv, Wproj):
    x2 = np.ascontiguousarray(x.reshape(T, C))
    xT = np.ascontiguousarray(x2.T)                       # [C, T]
    xT_a = np.ascontiguousarray(
        xT.reshape(NO, P, T).transpose(1, 0, 2)
    ).astype(NPBF16)

    # per-dim scale folded into Wq: 1/(rank*3) by level of (d % 64)
    colscale = np.where(np.arange(P) % D < 32, 1.0 / 96, 1.0 / 48).astype(
        np.float32
    )

    wproj_a = np.ascontiguousarray(
        Wproj.reshape(NO, P, C).transpose(1, 0, 2)
    ).astype(NPBF16)

    kp = np.arange(P)[:, None]
    qf = np.arange(QB)[None, :]
    masks = np.stack(
        [(qf >= kp + P * d).astype(np.float32) for d in range(4)], axis=0
    )
    masks_a = np.ascontiguousarray(masks.transpose(1, 0, 2)).astype(NPBF16)

    in_maps = []
    for c in range(NCORES):
        cs = slice(P * c, P * (c + 1))
        wq_c = Wqkv[:, cs] * colscale[None, :]
        wk_c = Wqkv[:, C : 2 * C][:, cs]
        wv_c = Wqkv[:, 2 * C :][:, cs]
        in_maps.append(
            {
                "xT": xT_a,
                "wq": np.ascontiguousarray(
                    wq_c.reshape(NO, P, P).transpose(1, 0, 2)
                ).astype(NPBF16),
                "wk": np.ascontiguousarray(
                    wk_c.reshape(NO, P, P).transpose(1, 0, 2)
                ).astype(NPBF16),
                "wv": np.ascontiguousarray(
                    wv_c.reshape(NO, P, P).transpose(1, 0, 2)
                ).astype(NPBF16),
                "wproj": wproj_a,
                "masks": masks_a,
            }
        )
    return in_maps


def kernel(x, Wqkv, Wproj, _trace=False):
    x = np.asarray(x, np.float32)
    Wqkv = np.asarray(Wqkv, np.float32)
    Wproj = np.asarray(Wproj, np.float32)

    if "nc" not in _CACHE:
        _CACHE["nc"] = _build()
    nc = _CACHE["nc"]

    in_maps = _prep_inputs(x, Wqkv, Wproj)
    res = run_bass_kernel_spmd(nc, in_maps, list(range(NCORES)), trace=_trace)
    _CACHE["last_result"] = res

    full = np.empty((T, C), np.float32)
    for c in range(NCORES):
        oc = res.results[c]["out"]  # [128, 2, 1024]
        full[2 * P * c : 2 * P * (c + 1)] = oc.transpose(1, 0, 2).reshape(
            2 * P, C
        )
    return full.reshape(1, T, C)
